# revision 4
# baseline (speedup 1.0000x reference)
"""Trainium2 Bass kernel for nn_MultiHeadAttention (T=2048, B=2, E=1024, H=16).

Sharding: head/data parallel over 8 cores. Core c handles batch b=c//4 and the
4 heads g=c%4 (head channels m in [g*256, (g+1)*256)). Each core is fully
independent (no collectives): it computes q/k/v projections for its head
slice, full attention for its heads, the partial row-parallel out-projection,
and the partial (4-head) sum of normalized attention weights in [s, t] layout.
The host gathers: sums the 4 partial out-projections per batch (+bo) and the 4
partial weight sums per batch (transposed to [t, s], /16 already folded in).

Device-side math per core (b, g):
  xqT/xkT [1024, 2048] = query/key[:, b, :].T (host layout prep)
  qT = (Wq_g xq^T + bq_g)  [256, 2048]  (bias via ACT per-partition bias)
  kT likewise; v [2048, 256] via xvT_aug (ones row folds bv into the matmul)
  per head pair: scoresT[s, t] = kT_h^T . qT_h (both heads packed in the
    128-row PE array); E = exp(scoresT / 8) (bf16)
  v-matmul: lhsT = [v_h | 1] (65 cols) -> psum rows 0..63 = (attn.v)^T
    unnormalized, row 64 = softmax denominator per t
  r = 1/denom; rank-1 PE broadcast of r/16 -> rbc [128, 512]
  outTn = psum[0:64] * rbc  (so outTn = (attn.v)^T / 16; Wo is host-scaled x16)
  avg += E * rbc  per head (bf16)  -> avgT partial [2048, 2048] (s-major)
  out_part[t, e] = sum_m outTn[m, t] * (16*Wo[e, m])
"""

import sys

if "/opt/trn_rl_repo" not in sys.path:
    sys.path.insert(0, "/opt/trn_rl_repo")

import numpy as np
import ml_dtypes

import concourse.bass as bass
import concourse.mybir as mybir
import concourse.tile as tile
from concourse import bacc
from concourse import bass_utils

N_CORES = 8
T = 2048          # query length
S = 2048          # key length
E = 1024          # embed dim
H = 16            # total heads
D = 64            # head dim
HLOC = 4          # heads per core
MLOC = HLOC * D   # 256 local head channels
KT = E // 128     # 8 contraction tiles
ST = S // 128     # 16 key-position tiles
TB = 512          # t block (psum bank)
NTB = T // TB     # 4
F32 = mybir.dt.float32
BF16 = mybir.dt.bfloat16

_CACHE = {}


def _emit(tc, aps):
    nc = tc.nc
    (xqT, xkT, xvT, wqT, wkT, wvT, bvrow, woT, bq, bk, out_part, avgT) = aps

    import contextlib
    with contextlib.ExitStack() as ctx:
        ec = ctx.enter_context
        xpool = ec(tc.tile_pool(name="xpool", bufs=9))
        wpool = ec(tc.tile_pool(name="wpool", bufs=9))
        wopool = ec(tc.tile_pool(name="wopool", bufs=2))
        persist = ec(tc.tile_pool(name="persist", bufs=1))
        vpool = ec(tc.tile_pool(name="vpool", bufs=16))
        epool = ec(tc.tile_pool(name="epool", bufs=17))
        apool = ec(tc.tile_pool(name="apool", bufs=1))
        obpool = ec(tc.tile_pool(name="obpool", bufs=3))
        small = ec(tc.tile_pool(name="small", bufs=4))
        psS = ec(tc.tile_pool(name="psS", bufs=4, space="PSUM"))
        psO = ec(tc.tile_pool(name="psO", bufs=2, space="PSUM"))

        # ---- constants ----
        ones16 = small.tile([1, 128], F32, tag="ones16", bufs=1)
        nc.gpsimd.memset(ones16[:], 1.0 / 16.0)
        bq_sb = [small.tile([128, 1], F32, tag=f"bq{m}", bufs=1, name=f"bq{m}")
                 for m in range(2)]
        bk_sb = [small.tile([128, 1], F32, tag=f"bk{m}", bufs=1, name=f"bk{m}")
                 for m in range(2)]
        for m in range(2):
            nc.sync.dma_start(bq_sb[m][:], bq[m * 128:(m + 1) * 128, :])
            nc.sync.dma_start(bk_sb[m][:], bk[m * 128:(m + 1) * 128, :])
        bv_sb = small.tile([1, MLOC], F32, tag="bv", bufs=1)
        nc.sync.dma_start(bv_sb[:], bvrow[:])

        # ---- weights (wq -> wk -> wv rotate through wpool) ----
        wo_sb = [wopool.tile([128, E], F32, tag="wo", name=f"wo{m}") for m in range(2)]
        for m in range(2):
            nc.sync.dma_start(wo_sb[m][:], woT[m * 128:(m + 1) * 128, :])

        # ---- persistent activation tensors ----
        qT = [persist.tile([128, T], F32, tag=f"qT{j}", name=f"qT{j}") for j in range(2)]
        kT = [persist.tile([128, T], F32, tag=f"kT{j}", name=f"kT{j}") for j in range(2)]
        outTn = [persist.tile([128, T], F32, tag=f"oT{j}", name=f"oT{j}") for j in range(2)]
        vaug = [vpool.tile([128, HLOC * (D + 1)], BF16, tag="vaug", name=f"vaug{s}")
                for s in range(ST)]

        def proj_qk(xT_dram, w_dram, dst, bias_sb):
            w_sb = [wpool.tile([128, MLOC], F32, tag="w", name="w") for k in range(KT)]
            for k in range(KT):
                nc.sync.dma_start(w_sb[k][:], w_dram[k * 128:(k + 1) * 128, :])
            for th in range(2):  # t halves
                x_sb = [xpool.tile([128, T // 2], F32, tag="x", name="x") for k in range(KT)]
                for k in range(KT):
                    nc.sync.dma_start(
                        x_sb[k][:], xT_dram[k * 128:(k + 1) * 128,
                                            th * (T // 2):(th + 1) * (T // 2)])
                for tt in range(2):  # 512 blocks in the half
                    for m in range(2):
                        ps = psS.tile([128, TB], F32, tag="ps")
                        for k in range(KT):
                            nc.tensor.matmul(
                                ps[:],
                                w_sb[k][:, m * 128:(m + 1) * 128],
                                x_sb[k][:, tt * TB:(tt + 1) * TB],
                                start=(k == 0), stop=(k == KT - 1))
                        nc.scalar.activation(
                            dst[m][:, th * (T // 2) + tt * TB:
                                   th * (T // 2) + (tt + 1) * TB],
                            ps[:], mybir.ActivationFunctionType.Identity,
                            bias=bias_sb[m][:])

        proj_qk(xqT, wqT, qT, bq_sb)
        proj_qk(xkT, wkT, kT, bk_sb)

        # ---- v projection (with bias via ones-row of xvT_aug) ----
        wv_sb = [wpool.tile([128, MLOC], F32, tag="w", name="w") for k in range(KT)]
        for k in range(KT):
            nc.sync.dma_start(wv_sb[k][:], wvT[k * 128:(k + 1) * 128, :])
        for sh in range(2):  # s halves
            x_sb = [xpool.tile([128, S // 2], F32, tag="x", name="x") for k in range(KT)]
            for k in range(KT):
                nc.sync.dma_start(
                    x_sb[k][:], xvT[k * 128:(k + 1) * 128,
                                    sh * (S // 2):(sh + 1) * (S // 2)])
            xo = small.tile([1, S // 2], F32, tag="xones")
            nc.sync.dma_start(xo[:], xvT[E:E + 1, sh * (S // 2):(sh + 1) * (S // 2)])
            for s8 in range(8):
                s = sh * 8 + s8
                ps = psS.tile([128, MLOC], F32, tag="ps")
                for k in range(KT):
                    nc.tensor.matmul(
                        ps[:], x_sb[k][:, s8 * 128:(s8 + 1) * 128], wv_sb[k][:],
                        start=(k == 0), stop=False)
                nc.tensor.matmul(
                    ps[:], xo[:, s8 * 128:(s8 + 1) * 128], bv_sb[:],
                    start=False, stop=True)
                va = vaug[s]
                va3 = va.rearrange("p (h x) -> p h x", h=HLOC)
                nc.vector.tensor_copy(
                    va3[:, :, 0:D],
                    ps.rearrange("p (h x) -> p h x", h=HLOC)[:])
                nc.gpsimd.memset(va3[:, :, D:D + 1], 1.0)

        # ---- attention ----
        for tb in range(NTB):
            tsl = slice(tb * TB, (tb + 1) * TB)
            avg_tiles = []
            for hp in range(2):
                j = hp
                po = [psO.tile([D + 1, TB], F32, tag=f"po{i}", name=f"po{i}")
                      for i in range(2)]
                Es = [[], []]
                for s in range(ST):
                    for i in range(2):
                        r0 = i * 64
                        psc = psS.tile([128, TB], F32, tag="ps", name="psc")
                        nc.tensor.matmul(
                            psc[:],
                            kT[j][r0:r0 + 64, s * 128:(s + 1) * 128],
                            qT[j][r0:r0 + 64, tsl])
                        Et = epool.tile([128, TB], BF16, tag=f"E{i}", name=f"E{i}")
                        nc.scalar.activation(
                            Et[:], psc[:], mybir.ActivationFunctionType.Exp,
                            scale=float(1.0 / np.sqrt(D)))
                        h = 2 * hp + i
                        nc.tensor.matmul(
                            po[i][:],
                            vaug[s][:, h * (D + 1):(h + 1) * (D + 1)],
                            Et[:], start=(s == 0), stop=(s == ST - 1))
                        Es[i].append(Et)
                # denominators -> r/16 broadcast
                rbf = []
                rbb = []
                for i in range(2):
                    r_t = small.tile([1, TB], F32, tag="r", name="r")
                    nc.vector.reciprocal(r_t[:], po[i][D:D + 1, :])
                    pb = psS.tile([128, TB], F32, tag="ps", name="pb")
                    nc.tensor.matmul(pb[:], ones16[:], r_t[:])
                    rf = small.tile([128, TB], F32, tag="rbf", name="rbf")
                    nc.vector.tensor_copy(rf[:], pb[:])
                    rb = small.tile([128, TB], BF16, tag="rbb", name="rbb")
                    nc.vector.tensor_copy(rb[:], rf[:])
                    rbf.append(rf)
                    rbb.append(rb)
                    nc.vector.tensor_mul(
                        outTn[j][i * 64 + 0:i * 64 + 64, tsl],
                        po[i][0:D, :], rf[0:D, :])
                for s in range(ST):
                    if hp == 0:
                        av = apool.tile([128, TB], BF16, tag=f"avg{s}",
                                        name=f"avg{s}", bufs=1)
                        avg_tiles.append(av)
                        nc.vector.tensor_mul(av[:], Es[0][s][:], rbb[0][:])
                    else:
                        av = avg_tiles[s]
                        nc.vector.tensor_mul(Es[0][s][:], Es[0][s][:], rbb[0][:])
                        nc.vector.tensor_add(av[:], av[:], Es[0][s][:])
                    nc.vector.tensor_mul(Es[1][s][:], Es[1][s][:], rbb[1][:])
                    nc.vector.tensor_add(av[:], av[:], Es[1][s][:])
            for s in range(ST):
                nc.sync.dma_start(avgT[s * 128:(s + 1) * 128, tsl], avg_tiles[s][:])

        # ---- out projection (row-parallel partial) ----
        for t16 in range(ST):
            for e in range(2):
                ps = psS.tile([128, TB], F32, tag="ps", name="pso")
                for m in range(2):
                    nc.tensor.matmul(
                        ps[:],
                        outTn[m][:, t16 * 128:(t16 + 1) * 128],
                        wo_sb[m][:, e * TB:(e + 1) * TB],
                        start=(m == 0), stop=(m == 1))
                ob = obpool.tile([128, TB], F32, tag="ob", name="ob")
                nc.vector.tensor_copy(ob[:], ps[:])
                nc.sync.dma_start(
                    out_part[t16 * 128:(t16 + 1) * 128, e * TB:(e + 1) * TB], ob[:])


def build():
    if "nc" in _CACHE:
        return _CACHE["nc"]
    nc = bacc.Bacc("TRN2", target_bir_lowering=False, debug=False,
                   num_devices=N_CORES)
    names = [
        ("xqT", [E, T], F32, "ExternalInput"),
        ("xkT", [E, S], F32, "ExternalInput"),
        ("xvT", [E + 1, S], F32, "ExternalInput"),
        ("wqT", [E, MLOC], F32, "ExternalInput"),
        ("wkT", [E, MLOC], F32, "ExternalInput"),
        ("wvT", [E, MLOC], F32, "ExternalInput"),
        ("bvrow", [1, MLOC], F32, "ExternalInput"),
        ("woT", [MLOC, E], F32, "ExternalInput"),
        ("bq", [MLOC, 1], F32, "ExternalInput"),
        ("bk", [MLOC, 1], F32, "ExternalInput"),
        ("out_part", [T, E], F32, "ExternalOutput"),
        ("avgT", [S, T], BF16, "ExternalOutput"),
    ]
    aps = [nc.dram_tensor(n, shp, dt, kind=k).ap() for (n, shp, dt, k) in names]
    with tile.TileContext(nc) as tc:
        _emit(tc, aps)
    nc.compile()
    _CACHE["nc"] = nc
    return nc


def make_in_maps(inputs):
    """inputs: full unsharded arrays keyed as in reference.setup_inputs()."""
    f = np.float32
    q_in = np.asarray(inputs["query"], f)
    k_in = np.asarray(inputs["key"], f)
    v_in = np.asarray(inputs["value"], f)
    Wq = np.asarray(inputs["Wq"], f)
    Wk = np.asarray(inputs["Wk"], f)
    Wv = np.asarray(inputs["Wv"], f)
    Wo = np.asarray(inputs["Wo"], f)
    bq = np.asarray(inputs["bq"], f)
    bk = np.asarray(inputs["bk"], f)
    bv = np.asarray(inputs["bv"], f)

    xT = {}
    for b in range(2):
        xT[("q", b)] = np.ascontiguousarray(q_in[:, b, :].T)
        xT[("k", b)] = np.ascontiguousarray(k_in[:, b, :].T)
        xv = np.ascontiguousarray(v_in[:, b, :].T)
        xT[("v", b)] = np.concatenate([xv, np.ones((1, S), f)], axis=0)
    wslices = {}
    for g in range(4):
        ms = slice(g * MLOC, (g + 1) * MLOC)
        wslices[("wqT", g)] = np.ascontiguousarray(Wq[ms, :].T)
        wslices[("wkT", g)] = np.ascontiguousarray(Wk[ms, :].T)
        wslices[("wvT", g)] = np.ascontiguousarray(Wv[ms, :].T)
        wslices[("bvrow", g)] = bv[ms].reshape(1, MLOC)
        wslices[("woT", g)] = np.ascontiguousarray(Wo[:, ms].T) * 16.0
        wslices[("bq", g)] = bq[ms].reshape(MLOC, 1)
        wslices[("bk", g)] = bk[ms].reshape(MLOC, 1)

    in_maps = []
    for c in range(N_CORES):
        b, g = divmod(c, 4)
        in_maps.append({
            "xqT": xT[("q", b)],
            "xkT": xT[("k", b)],
            "xvT": xT[("v", b)],
            "wqT": wslices[("wqT", g)],
            "wkT": wslices[("wkT", g)],
            "wvT": wslices[("wvT", g)],
            "bvrow": wslices[("bvrow", g)],
            "woT": wslices[("woT", g)],
            "bq": wslices[("bq", g)],
            "bk": wslices[("bk", g)],
        })
    return in_maps


def gather_outputs(results, inputs):
    bo = np.asarray(inputs["bo"], np.float32)
    out = np.empty((T, 2, E), np.float32)
    avg = np.empty((2, T, S), np.float32)
    for b in range(2):
        op = results[4 * b]["out_part"].astype(np.float32)
        for g in range(1, 4):
            op = op + results[4 * b + g]["out_part"].astype(np.float32)
        out[:, b, :] = op + bo[None, :]
        ag = results[4 * b]["avgT"].astype(np.float32)
        for g in range(1, 4):
            ag = ag + results[4 * b + g]["avgT"].astype(np.float32)
        avg[b] = ag.T
    return out, avg


def kernel(**inputs):
    nc = build()
    in_maps = make_in_maps(inputs)
    res = bass_utils.run_bass_kernel_spmd(nc, in_maps, core_ids=list(range(N_CORES)))
    return gather_outputs(res.results, inputs)


# revision 13
# speedup vs baseline: 201.2919x; 201.2919x over previous
"""Trainium2 Bass kernel for nn_MultiHeadAttention (T=2048, B=2, E=1024, H=16).

Sharding: head/data parallel over 8 cores. Core c handles batch b=c//4 and the
4 heads g=c%4 (head channels m in [g*256, (g+1)*256)). Each core is fully
independent (no collectives): it computes q/k/v projections for its head
slice, full attention for its heads, the partial row-parallel out-projection,
and the partial (4-head) sum of normalized attention weights in [s, t] layout.
The host gathers: sums the 4 partial out-projections per batch (+bo) and the 4
partial weight sums per batch (transposed to [t, s], /16 already folded in).

Device-side math per core (b, g):
  xqT/xkT [1024, 2048] = query/key[:, b, :].T (host layout prep)
  qT = (Wq_g xq^T + bq_g)  [256, 2048]  (bias via ACT per-partition bias)
  kT likewise; v [2048, 256] via xvT_aug (ones row folds bv into the matmul)
  per head pair: scoresT[s, t] = kT_h^T . qT_h (both heads packed in the
    128-row PE array); E = exp(scoresT / 8) (bf16)
  v-matmul: lhsT = [v_h | 1] (65 cols) -> psum rows 0..63 = (attn.v)^T
    unnormalized, row 64 = softmax denominator per t
  r = 1/denom; rank-1 PE broadcast of r/16 -> rbc [128, 512]
  outTn = psum[0:64] * rbc  (so outTn = (attn.v)^T / 16; Wo is host-scaled x16)
  avg += E * rbc  per head (bf16)  -> avgT partial [2048, 2048] (s-major)
  out_part[t, e] = sum_m outTn[m, t] * (16*Wo[e, m])
"""

import sys

if "/opt/trn_rl_repo" not in sys.path:
    sys.path.insert(0, "/opt/trn_rl_repo")

import numpy as np
import ml_dtypes

import concourse.bass as bass
import concourse.mybir as mybir
import concourse.tile as tile
from concourse import bacc
from concourse import bass_utils

N_CORES = 8
T = 2048          # query length
S = 2048          # key length
E = 1024          # embed dim
H = 16            # total heads
D = 64            # head dim
HLOC = 4          # heads per core
MLOC = HLOC * D   # 256 local head channels
KT = E // 128     # 8 contraction tiles
ST = S // 128     # 16 key-position tiles
TB = 512          # t block (1 psum bank; exp/avg ops run at FD=TB)
NTB = T // TB     # 4
F32 = mybir.dt.float32
BF16 = mybir.dt.bfloat16

_CACHE = {}


def _emit(tc, aps):
    nc = tc.nc
    (xqT, xkT, xvT, wqT, wkT, wvT, bvrow, woT, bq, bk, out_part, avgT) = aps

    import contextlib
    with contextlib.ExitStack() as ctx:
        ec = ctx.enter_context
        xpool = ec(tc.tile_pool(name="xpool", bufs=9))
        wpool = ec(tc.tile_pool(name="wpool", bufs=9))
        wopool = ec(tc.tile_pool(name="wopool", bufs=2))
        persist = ec(tc.tile_pool(name="persist", bufs=1))
        vpool = ec(tc.tile_pool(name="vpool", bufs=16))
        epool = ec(tc.tile_pool(name="epool", bufs=17))
        apool = ec(tc.tile_pool(name="apool", bufs=1))
        obpool = ec(tc.tile_pool(name="obpool", bufs=3))
        small = ec(tc.tile_pool(name="small", bufs=4))
        psS = ec(tc.tile_pool(name="psS", bufs=4, space="PSUM"))
        psO = ec(tc.tile_pool(name="psO", bufs=2, space="PSUM"))

        # ---- constants ----
        ones16 = small.tile([1, 128], F32, tag="ones16", bufs=1)
        nc.gpsimd.memset(ones16[:], 1.0 / 16.0)
        bq_sb = [small.tile([128, 1], F32, tag=f"bq{m}", bufs=1, name=f"bq{m}")
                 for m in range(2)]
        bk_sb = [small.tile([128, 1], F32, tag=f"bk{m}", bufs=1, name=f"bk{m}")
                 for m in range(2)]
        for m in range(2):
            nc.sync.dma_start(bq_sb[m][:], bq[m * 128:(m + 1) * 128, :])
            nc.sync.dma_start(bk_sb[m][:], bk[m * 128:(m + 1) * 128, :])
        bv_sb = small.tile([1, MLOC], F32, tag="bv", bufs=1)
        nc.sync.dma_start(bv_sb[:], bvrow[:])

        # ---- weights (wq -> wk -> wv rotate through wpool) ----
        wo_sb = [wopool.tile([128, E], F32, tag="wo", name=f"wo{m}") for m in range(2)]
        for m in range(2):
            nc.sync.dma_start(wo_sb[m][:], woT[m * 128:(m + 1) * 128, :])

        # ---- persistent activation tensors ----
        qT = [persist.tile([128, T], F32, tag=f"qT{j}", name=f"qT{j}") for j in range(2)]
        kT = [persist.tile([128, T], F32, tag=f"kT{j}", name=f"kT{j}") for j in range(2)]
        outTn = [persist.tile([128, T], F32, tag=f"oT{j}", name=f"oT{j}") for j in range(2)]
        vaug = [vpool.tile([128, HLOC * (D + 1)], BF16, tag="vaug", name=f"vaug{s}")
                for s in range(ST)]

        def proj_qk(xT_dram, w_dram, dst, bias_sb):
            w_sb = [wpool.tile([128, MLOC], F32, tag="w", name="w") for k in range(KT)]
            for k in range(KT):
                nc.sync.dma_start(w_sb[k][:], w_dram[k * 128:(k + 1) * 128, :])
            for th in range(2):  # t halves
                x_sb = [xpool.tile([128, T // 2], F32, tag="x", name="x") for k in range(KT)]
                for k in range(KT):
                    nc.sync.dma_start(
                        x_sb[k][:], xT_dram[k * 128:(k + 1) * 128,
                                            th * (T // 2):(th + 1) * (T // 2)])
                for tt in range(2):  # 512 blocks in the half
                    for m in range(2):
                        ps = psS.tile([128, 512], F32, tag="ps")
                        for k in range(KT):
                            nc.tensor.matmul(
                                ps[:],
                                w_sb[k][:, m * 128:(m + 1) * 128],
                                x_sb[k][:, tt * 512:(tt + 1) * 512],
                                start=(k == 0), stop=(k == KT - 1))
                        nc.scalar.activation(
                            dst[m][:, th * (T // 2) + tt * 512:
                                   th * (T // 2) + (tt + 1) * 512],
                            ps[:], mybir.ActivationFunctionType.Identity,
                            bias=bias_sb[m][:])

        proj_qk(xqT, wqT, qT, bq_sb)
        proj_qk(xkT, wkT, kT, bk_sb)

        # ---- v projection (with bias via ones-row of xvT_aug) ----
        wv_sb = [wpool.tile([128, MLOC], F32, tag="w", name="w") for k in range(KT)]
        for k in range(KT):
            nc.sync.dma_start(wv_sb[k][:], wvT[k * 128:(k + 1) * 128, :])
        for sh in range(2):  # s halves
            x_sb = [xpool.tile([128, S // 2], F32, tag="x", name="x") for k in range(KT)]
            for k in range(KT):
                nc.sync.dma_start(
                    x_sb[k][:], xvT[k * 128:(k + 1) * 128,
                                    sh * (S // 2):(sh + 1) * (S // 2)])
            xo = small.tile([1, S // 2], F32, tag="xones")
            nc.sync.dma_start(xo[:], xvT[E:E + 1, sh * (S // 2):(sh + 1) * (S // 2)])
            for s8 in range(8):
                s = sh * 8 + s8
                ps = psS.tile([128, MLOC], F32, tag="ps")
                for k in range(KT):
                    nc.tensor.matmul(
                        ps[:], x_sb[k][:, s8 * 128:(s8 + 1) * 128], wv_sb[k][:],
                        start=(k == 0), stop=False)
                nc.tensor.matmul(
                    ps[:], xo[:, s8 * 128:(s8 + 1) * 128], bv_sb[:],
                    start=False, stop=True)
                va = vaug[s]
                va3 = va.rearrange("p (h x) -> p h x", h=HLOC)
                nc.vector.tensor_copy(
                    va3[:, :, 0:D],
                    ps.rearrange("p (h x) -> p h x", h=HLOC)[:])
                nc.gpsimd.memset(va3[:, :, D:D + 1], 1.0)

        # ---- attention ----
        for tb in range(NTB):
            tsl = slice(tb * TB, (tb + 1) * TB)
            avg_tiles = []
            for hp in range(2):
                j = hp
                po = [psO.tile([D + 1, TB], F32, tag=f"po{i}", name=f"po{i}")
                      for i in range(2)]
                Es = [[], []]
                for s in range(ST):
                    for i in range(2):
                        r0 = i * 64
                        psc = psS.tile([128, TB], F32, tag="ps", name="psc")
                        for half in range(TB // 512):
                            nc.tensor.matmul(
                                psc[:, half * 512:(half + 1) * 512],
                                kT[j][r0:r0 + 64, s * 128:(s + 1) * 128],
                                qT[j][r0:r0 + 64,
                                      tb * TB + half * 512:tb * TB + (half + 1) * 512])
                        Et = epool.tile([128, TB], BF16, tag=f"E{i}", name=f"E{i}")
                        nc.scalar.activation(
                            Et[:], psc[:], mybir.ActivationFunctionType.Exp,
                            scale=float(1.0 / np.sqrt(D)))
                        h = 2 * hp + i
                        nc.tensor.matmul(
                            po[i][:],
                            vaug[s][:, h * (D + 1):(h + 1) * (D + 1)],
                            Et[:], start=(s == 0), stop=(s == ST - 1))
                        Es[i].append(Et)
                # denominators -> r/16 broadcast
                rbf = []
                rbb = []
                for i in range(2):
                    r_t = small.tile([1, TB], F32, tag="r", name="r")
                    nc.vector.reciprocal(r_t[:], po[i][D:D + 1, :])
                    pb = psS.tile([128, TB], F32, tag="ps", name="pb")
                    for half in range(TB // 512):
                        nc.tensor.matmul(pb[:, half * 512:(half + 1) * 512],
                                         ones16[:],
                                         r_t[:, half * 512:(half + 1) * 512])
                    rf = small.tile([128, TB], F32, tag="rbf", name="rbf")
                    nc.vector.tensor_copy(rf[:], pb[:])
                    rb = small.tile([128, TB], BF16, tag="rbb", name="rbb")
                    nc.vector.tensor_copy(rb[:], rf[:])
                    rbf.append(rf)
                    rbb.append(rb)
                    nc.vector.tensor_mul(
                        outTn[j][i * 64 + 0:i * 64 + 64, tsl],
                        po[i][0:D, :], rf[0:D, :])
                for s in range(ST):
                    if hp == 0:
                        av = apool.tile([128, TB], BF16, tag=f"avg{s}",
                                        name=f"avg{s}", bufs=1)
                        avg_tiles.append(av)
                        nc.vector.tensor_mul(av[:], Es[0][s][:], rbb[0][:])
                        nc.gpsimd.tensor_mul(Es[1][s][:], Es[1][s][:], rbb[1][:])
                        nc.vector.tensor_add(av[:], av[:], Es[1][s][:])
                    else:
                        av = avg_tiles[s]
                        nc.gpsimd.tensor_mul(Es[0][s][:], Es[0][s][:], rbb[0][:])
                        nc.vector.tensor_add(av[:], av[:], Es[0][s][:])
                        nc.gpsimd.tensor_mul(Es[1][s][:], Es[1][s][:], rbb[1][:])
                        nc.vector.tensor_add(av[:], av[:], Es[1][s][:])
            for s in range(ST):
                nc.sync.dma_start(avgT[s * 128:(s + 1) * 128, tsl], avg_tiles[s][:])

        # ---- out projection (row-parallel partial) ----
        for t16 in range(ST):
            for e in range(2):
                ps = psS.tile([128, 512], F32, tag="ps", name="pso")
                for m in range(2):
                    nc.tensor.matmul(
                        ps[:],
                        outTn[m][:, t16 * 128:(t16 + 1) * 128],
                        wo_sb[m][:, e * 512:(e + 1) * 512],
                        start=(m == 0), stop=(m == 1))
                ob = obpool.tile([128, 512], F32, tag="ob", name="ob")
                nc.vector.tensor_copy(ob[:], ps[:])
                nc.sync.dma_start(
                    out_part[t16 * 128:(t16 + 1) * 128, e * 512:(e + 1) * 512], ob[:])


def build():
    if "nc" in _CACHE:
        return _CACHE["nc"]
    nc = bacc.Bacc("TRN2", target_bir_lowering=False, debug=False,
                   num_devices=N_CORES)
    names = [
        ("xqT", [E, T], F32, "ExternalInput"),
        ("xkT", [E, S], F32, "ExternalInput"),
        ("xvT", [E + 1, S], F32, "ExternalInput"),
        ("wqT", [E, MLOC], F32, "ExternalInput"),
        ("wkT", [E, MLOC], F32, "ExternalInput"),
        ("wvT", [E, MLOC], F32, "ExternalInput"),
        ("bvrow", [1, MLOC], F32, "ExternalInput"),
        ("woT", [MLOC, E], F32, "ExternalInput"),
        ("bq", [MLOC, 1], F32, "ExternalInput"),
        ("bk", [MLOC, 1], F32, "ExternalInput"),
        ("out_part", [T, E], F32, "ExternalOutput"),
        ("avgT", [S, T], BF16, "ExternalOutput"),
    ]
    aps = [nc.dram_tensor(n, shp, dt, kind=k).ap() for (n, shp, dt, k) in names]
    with tile.TileContext(nc) as tc:
        _emit(tc, aps)
    nc.compile()
    _CACHE["nc"] = nc
    return nc


def make_in_maps(inputs):
    """inputs: full unsharded arrays keyed as in reference.setup_inputs()."""
    f = np.float32
    q_in = np.asarray(inputs["query"], f)
    k_in = np.asarray(inputs["key"], f)
    v_in = np.asarray(inputs["value"], f)
    Wq = np.asarray(inputs["Wq"], f)
    Wk = np.asarray(inputs["Wk"], f)
    Wv = np.asarray(inputs["Wv"], f)
    Wo = np.asarray(inputs["Wo"], f)
    bq = np.asarray(inputs["bq"], f)
    bk = np.asarray(inputs["bk"], f)
    bv = np.asarray(inputs["bv"], f)

    xT = {}
    for b in range(2):
        xT[("q", b)] = np.ascontiguousarray(q_in[:, b, :].T)
        xT[("k", b)] = np.ascontiguousarray(k_in[:, b, :].T)
        xv = np.ascontiguousarray(v_in[:, b, :].T)
        xT[("v", b)] = np.concatenate([xv, np.ones((1, S), f)], axis=0)
    wslices = {}
    for g in range(4):
        ms = slice(g * MLOC, (g + 1) * MLOC)
        wslices[("wqT", g)] = np.ascontiguousarray(Wq[ms, :].T)
        wslices[("wkT", g)] = np.ascontiguousarray(Wk[ms, :].T)
        wslices[("wvT", g)] = np.ascontiguousarray(Wv[ms, :].T)
        wslices[("bvrow", g)] = bv[ms].reshape(1, MLOC)
        wslices[("woT", g)] = np.ascontiguousarray(Wo[:, ms].T) * 16.0
        wslices[("bq", g)] = bq[ms].reshape(MLOC, 1)
        wslices[("bk", g)] = bk[ms].reshape(MLOC, 1)

    in_maps = []
    for c in range(N_CORES):
        b, g = divmod(c, 4)
        in_maps.append({
            "xqT": xT[("q", b)],
            "xkT": xT[("k", b)],
            "xvT": xT[("v", b)],
            "wqT": wslices[("wqT", g)],
            "wkT": wslices[("wkT", g)],
            "wvT": wslices[("wvT", g)],
            "bvrow": wslices[("bvrow", g)],
            "woT": wslices[("woT", g)],
            "bq": wslices[("bq", g)],
            "bk": wslices[("bk", g)],
        })
    return in_maps


def gather_outputs(results, inputs):
    bo = np.asarray(inputs["bo"], np.float32)
    out = np.empty((T, 2, E), np.float32)
    avg = np.empty((2, T, S), np.float32)
    for b in range(2):
        op = results[4 * b]["out_part"].astype(np.float32)
        for g in range(1, 4):
            op = op + results[4 * b + g]["out_part"].astype(np.float32)
        out[:, b, :] = op + bo[None, :]
        ag = results[4 * b]["avgT"].astype(np.float32)
        for g in range(1, 4):
            ag = ag + results[4 * b + g]["avgT"].astype(np.float32)
        avg[b] = ag.T
    return out, avg


def kernel(**inputs):
    nc = build()
    in_maps = make_in_maps(inputs)
    res = bass_utils.run_bass_kernel_spmd(nc, in_maps, core_ids=list(range(N_CORES)))
    return gather_outputs(res.results, inputs)
